# revision 10
# baseline (speedup 1.0000x reference)
"""Trainium2 Bass kernel for nn_DistanceTransform.

The reference's data-dependent while-loop collapses to a closed form:
    d(p)   = Chebyshev distance from p to the nearest seed
    S(p)   = sum over the 3x3 neighborhood (replicate-clamped) of
             w(dy,dx) * [d(q) < d(p)]
    out(p) = 0 if d(p)==0 else (d(p)-1) - h*ln(S(p))

The Chebyshev DT decomposes exactly into four 1D min-plus passes:
    D* = diagNE(diagSE(seed0))          (cost 1 per step along diagonals)
    d  = min(axisX(D*), axisY(D*))      (cost 1 per step along rows/cols)
Each 1D pass is a forward+backward `tensor_tensor_scan` along the free
dimension. Diagonal passes run in 45-degree-sheared layouts obtained via
DRAM staging buffers with mismatched read/write row pitches (per-row
contiguous DMA); the shear's transpose runs on the PE via identity
matmuls. S(p) uses PE banded matmuls for the row-shifted d fields and
DVE is_lt masks.

Data-parallel over B*C = 2 images: core b computes image b.
"""

import os
import numpy as np

import concourse.bacc as bacc
import concourse.mybir as mybir
from concourse.tile import TileContext
from concourse.masks import make_identity
from concourse.bass_utils import run_bass_kernel_spmd

F32 = mybir.dt.float32
F16 = mybir.dt.float16
AF = mybir.ActivationFunctionType
ALU = mybir.AluOpType

H = W = 256
HB = 2          # y blocks of 128
INF = 1536.0    # unreachable sentinel; stays fp16-exact under +k updates
H_PARAM = np.float32(0.35)
E1 = float(np.exp(np.float32(-1.0) / H_PARAM))          # axis-neighbor weight
EC = float(np.exp(np.float32(-np.sqrt(np.float32(2.0))) / H_PARAM))
LNSCALE = float(np.exp(np.float32(1.0) / H_PARAM))      # folds the (d-1) -1

# staging pitches (elements). Read pitch R, write pitch R-s gives a
# per-row shift of +s*y between write and read coordinates.
P1R = 516   # stage1 (f32): c = x + 255 - y  (write pitch 515, base 255)
P2R = 768   # stage2 (f16): c' = c + 2y - 255 (write pitch 770, read base 255)
P3R = 516   # stage3 (f16): x = c' - y        (write pitch 515)

N_CORES = 8


def _build_program():
    nc = bacc.Bacc("TRN2", target_bir_lowering=False, debug=False,
                   num_devices=N_CORES)
    img = nc.dram_tensor("img", [H, W], F32, kind="ExternalInput").ap()
    out = nc.dram_tensor("out", [H, W], F32, kind="ExternalOutput").ap()

    stage1 = nc.dram_tensor("stage1", [256 * P1R + 600], F32).ap()
    stage2 = nc.dram_tensor("stage2", [256 * P2R + 1200], F16).ap()
    stage3 = nc.dram_tensor("stage3", [256 * P3R + 600], F16).ap()

    dbg = {}
    if os.environ.get("DT_DEBUG"):
        for name, shape in [("dbg_d", [H, W]), ("dbg_dstar", [H, W]),
                            ("dbg_s", [H, W]), ("dbg_sk1", [H, 512]),
                            ("dbg_d1post", [512, 256]), ("dbg_mb", [H, 512]),
                            ("dbg_sk2", [H, 512])]:
            dbg[name] = nc.dram_tensor(name, shape, F32, kind="ExternalOutput").ap()

    with TileContext(nc) as tc:
        _emit(nc, tc, img, out, stage1, stage2, stage3, dbg)
    nc.compile()
    return nc


def _emit(nc, tc, img, out, stage1, stage2, stage3, dbg=None):
    dbg = dbg or {}
    import contextlib
    ctx = contextlib.ExitStack()
    const = ctx.enter_context(tc.tile_pool(name="const", bufs=1))
    work = ctx.enter_context(tc.tile_pool(name="work", bufs=1))
    psum = ctx.enter_context(tc.tile_pool(name="psum", bufs=4, space="PSUM"))
    psum2 = ctx.enter_context(tc.tile_pool(name="psum2", bufs=2, space="PSUM"))

    # ---------------- constants / init (off critical path) ----------------
    ident16 = const.tile([128, 128], F16)
    make_identity(nc, ident16[:])

    ones16 = const.tile([128, 512], F16)
    nc.vector.memset(ones16[:], 1.0)

    zero32 = const.tile([128, 1033], F32)
    nc.vector.memset(zero32[:], 0.0)

    inf16 = const.tile([128, 1540], F16)
    nc.vector.memset(inf16[:], INF)

    # prefill stage1 with zeros (maps to INF after the seed transform)
    n1 = 256 * P1R
    assert n1 % 128 == 0
    nc.sync.dma_start(
        out=stage1[:n1].rearrange("(p f) -> p f", p=128),
        in_=zero32[:, : n1 // 128])
    # prefill stage2 with INF
    n2 = 256 * P2R
    assert n2 % 128 == 0
    nc.sync.dma_start(
        out=stage2[:n2].rearrange("(p f) -> p f", p=128),
        in_=inf16[:, : n2 // 128])

    # sanitize mask MB (int8): 1 where c' is OUTSIDE [y, y+255], else 0.
    # y = 128*b + p, c' = free index in [0, 512). Start at 0; fill 1 on the
    # false side of each predicate (the two invalid half-planes compose).
    V16 = const.tile([128, HB, 512], F16)
    nc.vector.memset(V16[:], 1.0)
    for b in range(HB):
        nc.gpsimd.affine_select(
            out=V16[:, b, :], in_=V16[:, b, :], compare_op=ALU.is_ge,
            fill=0.0, base=-128 * b, pattern=[[1, 512]], channel_multiplier=-1)
        nc.gpsimd.affine_select(
            out=V16[:, b, :], in_=V16[:, b, :], compare_op=ALU.is_ge,
            fill=0.0, base=255 + 128 * b, pattern=[[-1, 512]],
            channel_multiplier=1)
    MBf = const.tile([128, HB, 512], F16)
    nc.scalar.activation(MBf[:], V16[:], AF.Copy, bias=1.0, scale=-1.0)
    MB = const.tile([128, HB, 512], mybir.dt.int16)
    nc.vector.tensor_copy(out=MB[:], in_=MBf[:])

    if "dbg_mb" in dbg:
        t = work.tile([128, HB, 512], F32, tag="dbgmb")
        nc.vector.tensor_copy(out=t[:], in_=MB[:])
        for b in range(HB):
            nc.sync.dma_start(out=dbg["dbg_mb"][128*b:128*(b+1), :], in_=t[:, b, :])
    # banded matrices for row shifts (lhsT layout: [k, m] = in-row k,
    # out-row m). up: out[m] = in[m-1] (replicate top); down: out[m]=in[m+1].
    def band(tile_ap, diag_base, corner=None):
        nc.gpsimd.memset(tile_ap, 0.0)
        nc.gpsimd.affine_select(
            out=tile_ap, in_=tile_ap, compare_op=ALU.not_equal, fill=1.0,
            base=diag_base, pattern=[[-1, 128]], channel_multiplier=1)
        if corner == "tl":  # [0, 0]
            nc.gpsimd.affine_select(
                out=tile_ap, in_=tile_ap, compare_op=ALU.not_equal, fill=1.0,
                base=0, pattern=[[1, 128]], channel_multiplier=1)
        elif corner == "br":  # [127, 127]
            nc.gpsimd.affine_select(
                out=tile_ap, in_=tile_ap, compare_op=ALU.not_equal, fill=1.0,
                base=-254, pattern=[[1, 128]], channel_multiplier=1)

    supA = const.tile([128, 128], F16)   # block0 up: k=m-1 + corner [0,0]
    band(supA[:], 1, corner="tl")
    supB = const.tile([128, 128], F16)   # block1 up: k=m-1 only
    band(supB[:], 1)
    sdnA = const.tile([128, 128], F16)   # block0 down: k=m+1 only
    band(sdnA[:], -1)
    sdnB = const.tile([128, 128], F16)   # block1 down: k=m+1 + corner [127,127]
    band(sdnB[:], -1, corner="br")
    # cross-block single entries: e_up[k=127, m=0], e_dn[k=0, m=127]
    e_up = const.tile([128, 128], F16)
    nc.gpsimd.memset(e_up[:], 0.0)
    nc.gpsimd.affine_select(
        out=e_up[:], in_=e_up[:], compare_op=ALU.not_equal, fill=1.0,
        base=127, pattern=[[1, 128]], channel_multiplier=-1)
    e_dn = const.tile([128, 128], F16)
    nc.gpsimd.memset(e_dn[:], 0.0)
    nc.gpsimd.affine_select(
        out=e_dn[:], in_=e_dn[:], compare_op=ALU.not_equal, fill=1.0,
        base=127, pattern=[[-1, 128]], channel_multiplier=1)

    # image tile for the final stage (off critical path)
    img32 = work.tile([128, HB, 256], F32)
    for b in range(HB):
        nc.sync.dma_start(out=img32[:, b, :], in_=img[128 * b:128 * (b + 1), :])

    # ---------------- shear A: img -> stage1 (c = x + 255 - y) ------------
    # write(y, x) at 255 + y*(P1R-1) + x ; read(y, c) at y*P1R + c
    bandA = stage1[255: 255 + 256 * (P1R - 1)].rearrange(
        "(y f) -> y f", f=P1R - 1)[:, :W]
    nc.sync.dma_start(out=bandA, in_=img[:, :])

    skraw = work.tile([128, HB, 512], F32)
    for b in range(HB):
        rd = stage1[128 * b * P1R: (128 * b + 128) * P1R].rearrange(
            "(y f) -> y f", f=P1R)[:, :512]
        nc.sync.dma_start(out=skraw[:, b, :], in_=rd)

    # seed transform: 0 -> INF, 1 -> 0   (pads: 0 -> INF, correct)
    sk1 = work.tile([128, HB, 512], F16)
    nc.scalar.activation(sk1[:], skraw[:], AF.Copy, bias=INF, scale=-INF)

    if "dbg_sk1" in dbg:
        t = work.tile([128, HB, 512], F32, tag="dbgt")
        nc.vector.tensor_copy(out=t[:], in_=sk1[:])
        for b in range(HB):
            nc.sync.dma_start(out=dbg["dbg_sk1"][128*b:128*(b+1), :], in_=t[:, b, :])

    # ---------------- transpose to D1 [c | y] ------------------------------
    def transpose_2to4(src, dst):
        # src [128, 2, 512] (y-part) -> dst [128, 4, 256] (c-part)
        k = 0
        for cb in range(4):
            for yb in range(2):
                pt = psum.tile([128, 128], F16, tag="tp")
                nc.tensor.transpose(
                    pt[:], src[:, yb, 128 * cb:128 * (cb + 1)], ident16[:])
                eng = nc.scalar if (k % 2 == 0) else nc.vector
                if eng is nc.scalar:
                    eng.copy(out=dst[:, cb, 128 * yb:128 * (yb + 1)], in_=pt[:])
                else:
                    eng.tensor_copy(out=dst[:, cb, 128 * yb:128 * (yb + 1)],
                                    in_=pt[:])
                k += 1

    def transpose_4to2(src, dst):
        # src [128, 4, 256] (c-part) -> dst [128, 2, 512] (y-part)
        k = 0
        for yb in range(2):
            for cb in range(4):
                pt = psum.tile([128, 128], F16, tag="tp")
                nc.tensor.transpose(
                    pt[:], src[:, cb, 128 * yb:128 * (yb + 1)], ident16[:])
                eng = nc.scalar if (k % 2 == 0) else nc.vector
                if eng is nc.scalar:
                    eng.copy(out=dst[:, yb, 128 * cb:128 * (cb + 1)], in_=pt[:])
                else:
                    eng.tensor_copy(out=dst[:, yb, 128 * cb:128 * (cb + 1)],
                                    in_=pt[:])
                k += 1

    d1 = work.tile([128, 4, 256], F16)
    transpose_2to4(sk1, d1)

    # ---------------- diagonal SE pass (scan fwd + bwd along y) ------------
    def minplus_pass(arr, nblk, width):
        for cb in range(nblk):
            tmp = work.tile([128, width], F16, tag="scantmp")
            nc.vector.tensor_tensor_scan(
                out=tmp[:], data0=ones16[:, :width], data1=arr[:, cb, :],
                initial=INF, op0=ALU.add, op1=ALU.min)
            nc.vector.tensor_tensor_scan(
                out=arr[:, cb, ::-1], data0=ones16[:, :width],
                data1=tmp[:, ::-1], initial=INF, op0=ALU.add, op1=ALU.min)

    minplus_pass(d1, 4, 256)

    if "dbg_d1post" in dbg:
        t = work.tile([128, 4, 256], F32, tag="dbgt2")
        nc.vector.tensor_copy(out=t[:], in_=d1[:])
        for cb in range(4):
            nc.sync.dma_start(out=dbg["dbg_d1post"][128*cb:128*(cb+1), :], in_=t[:, cb, :])

    # ---------------- back to [y | c], shear B to c' = x + y ---------------
    sk1b = work.tile([128, HB, 512], F16)
    transpose_4to2(d1, sk1b)

    # write(y, c) at (P2R+2)*y + c ; read(y, c') at P2R*y + 255 + c'
    for b in range(HB):
        wr = stage2[128 * b * (P2R + 2): (128 * b + 128) * (P2R + 2)].rearrange(
            "(y f) -> y f", f=P2R + 2)[:, :512]
        nc.sync.dma_start(out=wr, in_=sk1b[:, b, :])
    sk2 = work.tile([128, HB, 512], F16)
    for b in range(HB):
        rd = stage2[255 + 128 * b * P2R: 255 + (128 * b + 128) * P2R].rearrange(
            "(y f) -> y f", f=P2R)[:, :512]
        nc.sync.dma_start(out=sk2[:, b, :], in_=rd)

    # sanitize shear-polluted pads back to INF
    nc.vector.copy_predicated(
        out=sk2[:], mask=MB[:],
        data=inf16[:, :1024].rearrange("p (b f) -> p b f", b=HB))

    if "dbg_sk2" in dbg:
        t = work.tile([128, HB, 512], F32, tag="dbgsk2")
        nc.vector.tensor_copy(out=t[:], in_=sk2[:])
        for b in range(HB):
            nc.sync.dma_start(out=dbg["dbg_sk2"][128*b:128*(b+1), :], in_=t[:, b, :])

    # ---------------- transpose, diagonal NE pass, transpose back ----------
    d2 = work.tile([128, 4, 256], F16)
    transpose_2to4(sk2, d2)
    minplus_pass(d2, 4, 256)
    sk2b = work.tile([128, HB, 512], F16)
    transpose_4to2(d2, sk2b)

    # ---------------- unshear C: back to image layout ----------------------
    # write(y, c') at (P3R-1)*y + c' ; read(y, x) at P3R*y + x  (x = c'-y)
    for b in range(HB):
        wr = stage3[128 * b * (P3R - 1): (128 * b + 128) * (P3R - 1)].rearrange(
            "(y f) -> y f", f=P3R - 1)[:, :512]
        nc.sync.dma_start(out=wr, in_=sk2b[:, b, :])
    dstar = work.tile([128, HB, 256], F16)
    for b in range(HB):
        rd = stage3[128 * b * P3R: (128 * b + 128) * P3R].rearrange(
            "(y f) -> y f", f=P3R)[:, :256]
        nc.sync.dma_start(out=dstar[:, b, :], in_=rd)

    if "dbg_dstar" in dbg:
        t = work.tile([128, HB, 256], F32, tag="dbgt3")
        nc.vector.tensor_copy(out=t[:], in_=dstar[:])
        for b in range(HB):
            nc.sync.dma_start(out=dbg["dbg_dstar"][128*b:128*(b+1), :], in_=t[:, b, :])

    # ---------------- axis passes: d = min(axisX(D*), axisY(D*)) -----------
    dx = work.tile([128, HB, 256], F16)
    for b in range(HB):
        tmp = work.tile([128, 256], F16, tag="scantmp2")
        nc.vector.tensor_tensor_scan(
            out=tmp[:], data0=ones16[:, :256], data1=dstar[:, b, :],
            initial=INF, op0=ALU.add, op1=ALU.min)
        nc.vector.tensor_tensor_scan(
            out=dx[:, b, ::-1], data0=ones16[:, :256], data1=tmp[:, ::-1],
            initial=INF, op0=ALU.add, op1=ALU.min)

    # transpose D* to [x | y]
    dstT = work.tile([128, HB, 256], F16)
    k = 0
    for xb in range(2):
        for yb in range(2):
            pt = psum.tile([128, 128], F16, tag="tp")
            nc.tensor.transpose(
                pt[:], dstar[:, yb, 128 * xb:128 * (xb + 1)], ident16[:])
            eng = nc.scalar if (k % 2 == 0) else nc.vector
            if eng is nc.scalar:
                eng.copy(out=dstT[:, xb, 128 * yb:128 * (yb + 1)], in_=pt[:])
            else:
                eng.tensor_copy(out=dstT[:, xb, 128 * yb:128 * (yb + 1)],
                                in_=pt[:])
            k += 1
    dyT = work.tile([128, HB, 256], F16)
    for b in range(HB):
        tmp = work.tile([128, 256], F16, tag="scantmp3")
        nc.vector.tensor_tensor_scan(
            out=tmp[:], data0=ones16[:, :256], data1=dstT[:, b, :],
            initial=INF, op0=ALU.add, op1=ALU.min)
        nc.vector.tensor_tensor_scan(
            out=dyT[:, b, ::-1], data0=ones16[:, :256], data1=tmp[:, ::-1],
            initial=INF, op0=ALU.add, op1=ALU.min)
    dy = work.tile([128, HB, 256], F16)
    k = 0
    for yb in range(2):
        for xb in range(2):
            pt = psum.tile([128, 128], F16, tag="tp")
            nc.tensor.transpose(
                pt[:], dyT[:, xb, 128 * yb:128 * (yb + 1)], ident16[:])
            eng = nc.scalar if (k % 2 == 0) else nc.vector
            if eng is nc.scalar:
                eng.copy(out=dy[:, yb, 128 * xb:128 * (xb + 1)], in_=pt[:])
            else:
                eng.tensor_copy(out=dy[:, yb, 128 * xb:128 * (xb + 1)],
                                in_=pt[:])
            k += 1

    d16 = work.tile([128, HB, 256], F16)
    nc.vector.tensor_tensor(out=d16[:], in0=dx[:], in1=dy[:], op=ALU.min)

    if "dbg_d" in dbg:
        t = work.tile([128, HB, 256], F32, tag="dbgt4")
        nc.vector.tensor_copy(out=t[:], in_=d16[:])
        for b in range(HB):
            nc.sync.dma_start(out=dbg["dbg_d"][128*b:128*(b+1), :], in_=t[:, b, :])

    # ---------------- S stage --------------------------------------------
    # row-shifted fields via PE banded matmuls
    pup = psum2.tile([128, HB, 256], F32, tag="pup")
    nc.tensor.matmul(pup[:, 0, :], supA[:], d16[:, 0, :], start=True, stop=True)
    nc.tensor.matmul(pup[:, 1, :], supB[:], d16[:, 1, :], start=True, stop=False)
    nc.tensor.matmul(pup[:, 1, :], e_up[:], d16[:, 0, :], start=False, stop=True)
    pdn = psum2.tile([128, HB, 256], F32, tag="pdn")
    nc.tensor.matmul(pdn[:, 0, :], sdnA[:], d16[:, 0, :], start=True, stop=False)
    nc.tensor.matmul(pdn[:, 0, :], e_dn[:], d16[:, 1, :], start=False, stop=True)
    nc.tensor.matmul(pdn[:, 1, :], sdnB[:], d16[:, 1, :], start=True, stop=True)
    up16 = work.tile([128, HB, 256], F16)
    nc.scalar.copy(out=up16[:], in_=pup[:])
    dn16 = work.tile([128, HB, 256], F16)
    nc.vector.tensor_copy(out=dn16[:], in_=pdn[:])

    # masks: GA = axis taps [up, down, left, right]; GD = diagonal taps
    GA = work.tile([128, 4, HB, 256], F16)
    nc.vector.memset(GA[:], 0.0)
    GD = work.tile([128, 4, HB, 256], F16)
    nc.vector.memset(GD[:], 0.0)

    nc.vector.tensor_tensor(out=GA[:, 0], in0=up16[:], in1=d16[:], op=ALU.is_lt)
    nc.vector.tensor_tensor(out=GA[:, 1], in0=dn16[:], in1=d16[:], op=ALU.is_lt)
    for b in range(HB):
        nc.vector.tensor_tensor(          # left (0,-1), x>=1
            out=GA[:, 2, b, 1:], in0=d16[:, b, :-1], in1=d16[:, b, 1:],
            op=ALU.is_lt)
        nc.vector.tensor_tensor(          # right (0,+1), x<=254
            out=GA[:, 3, b, :-1], in0=d16[:, b, 1:], in1=d16[:, b, :-1],
            op=ALU.is_lt)
        nc.vector.tensor_tensor(          # up-left (-1,-1)
            out=GD[:, 0, b, 1:], in0=up16[:, b, :-1], in1=d16[:, b, 1:],
            op=ALU.is_lt)
        nc.vector.tensor_tensor(          # up-right (-1,+1)
            out=GD[:, 1, b, :-1], in0=up16[:, b, 1:], in1=d16[:, b, :-1],
            op=ALU.is_lt)
        nc.vector.tensor_tensor(          # down-left (+1,-1)
            out=GD[:, 2, b, 1:], in0=dn16[:, b, :-1], in1=d16[:, b, 1:],
            op=ALU.is_lt)
        nc.vector.tensor_tensor(          # down-right (+1,+1)
            out=GD[:, 3, b, :-1], in0=dn16[:, b, 1:], in1=d16[:, b, :-1],
            op=ALU.is_lt)
    # x-border clamp: diagonal taps collapse onto the vertical taps
    for b in range(HB):
        nc.vector.tensor_copy(out=GD[:, 0, b, 0:1], in_=GA[:, 0, b, 0:1])
        nc.vector.tensor_copy(out=GD[:, 2, b, 0:1], in_=GA[:, 1, b, 0:1])
        nc.vector.tensor_copy(out=GD[:, 1, b, 255:256], in_=GA[:, 0, b, 255:256])
        nc.vector.tensor_copy(out=GD[:, 3, b, 255:256], in_=GA[:, 1, b, 255:256])

    # tree sums (exact small ints in fp16)
    sa01 = work.tile([128, HB, 256], F16)
    nc.vector.tensor_tensor(out=sa01[:], in0=GA[:, 0], in1=GA[:, 1], op=ALU.add)
    sa23 = work.tile([128, HB, 256], F16)
    nc.vector.tensor_tensor(out=sa23[:], in0=GA[:, 2], in1=GA[:, 3], op=ALU.add)
    SA = work.tile([128, HB, 256], F16)
    nc.vector.tensor_tensor(out=SA[:], in0=sa01[:], in1=sa23[:], op=ALU.add)
    sd01 = work.tile([128, HB, 256], F16)
    nc.vector.tensor_tensor(out=sd01[:], in0=GD[:, 0], in1=GD[:, 1], op=ALU.add)
    sd23 = work.tile([128, HB, 256], F16)
    nc.vector.tensor_tensor(out=sd23[:], in0=GD[:, 2], in1=GD[:, 3], op=ALU.add)
    SD = work.tile([128, HB, 256], F16)
    nc.vector.tensor_tensor(out=SD[:], in0=sd01[:], in1=sd23[:], op=ALU.add)

    # S = E1*SA + EC*SD + seed   (f32)
    sa32 = work.tile([128, HB, 256], F32)
    nc.scalar.activation(sa32[:], SA[:], AF.Copy, bias=0.0, scale=E1)
    s32 = work.tile([128, HB, 256], F32)
    nc.vector.scalar_tensor_tensor(
        out=s32[:], in0=SD[:], scalar=EC, in1=sa32[:],
        op0=ALU.mult, op1=ALU.add)
    if "dbg_s" in dbg:
        for b in range(HB):
            nc.sync.dma_start(out=dbg["dbg_s"][128*b:128*(b+1), :], in_=s32[:, b, :])
    sg = work.tile([128, HB, 256], F32)
    nc.vector.tensor_tensor(out=sg[:], in0=s32[:], in1=img32[:], op=ALU.add)

    # out = d - h*ln(LNSCALE * Sg), zeroed at seeds
    lnv = work.tile([128, HB, 256], F32)
    nc.scalar.activation(lnv[:], sg[:], AF.Ln, bias=0.0, scale=LNSCALE)
    d32 = work.tile([128, HB, 256], F32)
    nc.scalar.copy(out=d32[:], in_=d16[:])
    outp = work.tile([128, HB, 256], F32)
    nc.vector.scalar_tensor_tensor(
        out=outp[:], in0=lnv[:], scalar=float(-H_PARAM), in1=d32[:],
        op0=ALU.mult, op1=ALU.add)
    seed8 = work.tile([128, HB, 256], mybir.dt.int8)
    nc.vector.tensor_copy(out=seed8[:], in_=img32[:])
    nc.vector.copy_predicated(
        out=outp[:], mask=seed8[:],
        data=zero32[:, :512].rearrange("p (b f) -> p b f", b=HB))

    for b in range(HB):
        nc.sync.dma_start(out=out[128 * b:128 * (b + 1), :], in_=outp[:, b, :])

    ctx.close()


_NC_CACHE = None


def _get_nc():
    global _NC_CACHE
    if _NC_CACHE is None:
        _NC_CACHE = _build_program()
    return _NC_CACHE


def kernel(image: np.ndarray) -> np.ndarray:
    """image: (2, 1, 256, 256) float32 -> (2, 1, 256, 256) float32."""
    B, C, Himg, Wimg = image.shape
    flat = np.ascontiguousarray(image.reshape(B * C, Himg, Wimg).astype(np.float32))
    n_units = flat.shape[0]
    nc = _get_nc()
    in_maps = [{"img": flat[i % n_units]} for i in range(N_CORES)]
    res = run_bass_kernel_spmd(nc, in_maps, core_ids=list(range(N_CORES)))
    outs = [res.results[i]["out"] for i in range(n_units)]
    return np.stack(outs).reshape(B, C, Himg, Wimg).astype(image.dtype)


if __name__ == "__main__":
    # quick simulator self-test on image 0
    from concourse.bass_interp import CoreSim
    import jax
    cpu = jax.devices("cpu")[0]
    with jax.default_device(cpu):
        import reference as R
        inputs = R.setup_inputs()
        img_np = np.asarray(inputs["image"]).reshape(2, 256, 256)
        expected = np.asarray(R.reference(**inputs)).reshape(2, 256, 256)
    print("reference done", flush=True)
    nc = _get_nc()
    print("program built", flush=True)
    sim = CoreSim(nc)
    sim.tensor("img")[:] = img_np[0]
    sim.simulate()
    got = sim.tensor("out").copy()
    err = np.abs(got - expected[0])
    rel = err.max() / (np.abs(expected[0]).max() + 1e-9)
    print("sim image0: max abs err", err.max(), "rel", rel)
